# revision 1
# baseline (speedup 1.0000x reference)
"""CoordinateDensification kernel for 8 TRN2 NeuronCores.

Reference semantics: expand 500k int32 coords [N,4] (cols 0-2 in [0,256),
col 3 == 0) by the 27 offsets {-2,0,2}^3 (stride 2), then sorted row-dedup
padded with INT32_MAX to [N*27, 4].

Device algorithm (SPMD over 8 cores, sharded by 33 dilated z-planes/core):
bit-packed occupancy slab in, 3D binary dilation by {-2,0,2}^3 on device,
packed bitmask out.  ~17 large instructions per core (vs ~2900 in the
scatter-based predecessor; this environment's cost is dominated by
per-instruction overhead, not bytes):
  - host marshals coords into a bit-packed occupancy slab
    gridin[265, 37*33] per core: bit (x+4)&7 of byte
    [y+4, (z+4-33c)*33 + (x+4)>>3] (little bit order; rows 260+ zero).
    The +4 origins guarantee rows/planes 0..3 and bits 0..3 of each
    plane's first byte are empty, making every cross-byte / cross-block
    carry in the shifted ORs below provably zero.
  - y-rows mapped row = p + 65*v (65 partitions, v in {0..3} free
    blocks): the whole 260-row slab is ONE pipeline.  y-dilation: the
    slab is loaded 3x at row offsets +0/+2/+4 (DVE cannot read at a
    partition offset; DMA can) and OR'd.
  - x-dilation: fused shift-OR ops in packed-bit space
    (U | U>>2 | U>>4 | next_byte<<6 | next_byte<<4).
  - z-dilation: shifted ORs at 66/132-byte strides.
  - output dil[z*260+Y, 33] is already the packed bitmask; bitmask cell
    order == lexicographic row order of the reference output, so no sort
    is ever needed.  Host unpacks + pads (gather/unshard).
"""
import sys
sys.path.insert(0, '/opt/trn_rl_repo')
import numpy as np

N = 500000
ZPL = 33                 # dilated planes owned per core
ZB = 37                  # z-slab planes per core (33 + 2 halo each side)
PB = 33                  # packed bytes per plane-row (264 bits)
ROWB = ZB * PB           # 1221 bytes per y-row
ROWS = 265               # 260 y-rows + 4 zero rows for the +2/+4 reads
NP = 65                  # partitions used; row = p + 65*v
NV = 4                   # row blocks per partition (65*4 == 260)
F3 = NV * ROWB           # 4884: per-partition free size
BUSE = ZPL * PB          # 1089: bytes per row after z-dilate
FILL = np.int32(np.iinfo(np.int32).max)
OUT_ROWS = N * 27

_NC_CACHE = {}


def _build_nc(num_devices=8, repeats=1):
    """repeats > 1 duplicates the body back-to-back in one NEFF; used by
    test.py to measure per-iteration HW time as a wall-clock delta."""
    key = ("nc", num_devices, repeats)
    if key in _NC_CACHE:
        return _NC_CACHE[key]
    import concourse.bass as bass
    import concourse.bacc as bacc
    import concourse.tile as tile
    from concourse import mybir

    u8 = mybir.dt.uint8
    OR = mybir.AluOpType.bitwise_or
    SHR = mybir.AluOpType.logical_shift_right
    SHL = mybir.AluOpType.logical_shift_left
    F = ROWB

    nc = bacc.Bacc("TRN2", target_bir_lowering=False, num_devices=num_devices)
    gridin = nc.dram_tensor("gridin", [ROWS, ROWB], u8, kind="ExternalInput")
    dil = nc.dram_tensor("dil", [ZPL * 260, 33], u8, kind="ExternalOutput")

    with tile.TileContext(nc) as tc:
        with tc.tile_pool(name="sbuf", bufs=2) as pool:
            # shift amounts as u8 tiles: bitvec stt ops reject float
            # immediates and need scalars of the operand dtype
            c2 = pool.tile([128, 1], u8, tag="c2")
            nc.vector.memset(c2[:], 2)
            c4 = pool.tile([128, 1], u8, tag="c4")
            nc.vector.memset(c4[:], 4)
            c6 = pool.tile([128, 1], u8, tag="c6")
            nc.vector.memset(c6[:], 6)

            for _rep in range(repeats):
                # rows p+65v per partition; +2/+4 y-shifts via DMA offset
                R0 = pool.tile([128, F3], u8, tag="R0")
                nc.sync.dma_start(
                    out=R0[:NP, :],
                    in_=bass.AP(gridin, 0, [[F, NP], [NP * F, NV], [1, F]]))
                R2 = pool.tile([128, F3], u8, tag="R2")
                nc.scalar.dma_start(
                    out=R2[:NP, :],
                    in_=bass.AP(gridin, 2 * F,
                                [[F, NP], [NP * F, NV], [1, F]]))
                R4 = pool.tile([128, F3], u8, tag="R4")
                nc.gpsimd.dma_start(
                    out=R4[:NP, :],
                    in_=bass.AP(gridin, 4 * F,
                                [[F, NP], [NP * F, NV], [1, F]]))
                # Y
                U = pool.tile([128, F3], u8, tag="U")
                nc.vector.tensor_tensor(
                    out=U[:NP, :], in0=R0[:NP, :], in1=R2[:NP, :], op=OR)
                nc.vector.tensor_tensor(
                    out=U[:NP, :], in0=U[:NP, :], in1=R4[:NP, :], op=OR)
                # X: T = U | U>>2 | U>>4 | next<<6 | next<<4
                T = pool.tile([128, F3], u8, tag="T")
                nc.vector.scalar_tensor_tensor(
                    out=T[:NP, :], in0=U[:NP, :], scalar=c2[:NP, :],
                    in1=U[:NP, :], op0=SHR, op1=OR)
                nc.vector.scalar_tensor_tensor(
                    out=T[:NP, :], in0=U[:NP, :], scalar=c4[:NP, :],
                    in1=T[:NP, :], op0=SHR, op1=OR)
                nc.vector.scalar_tensor_tensor(
                    out=T[:NP, 0:F3 - 1], in0=U[:NP, 1:F3], scalar=c6[:NP, :],
                    in1=T[:NP, 0:F3 - 1], op0=SHL, op1=OR)
                nc.vector.scalar_tensor_tensor(
                    out=T[:NP, 0:F3 - 1], in0=U[:NP, 1:F3], scalar=c4[:NP, :],
                    in1=T[:NP, 0:F3 - 1], op0=SHL, op1=OR)
                # Z: B[k] = T[k] | T[k+66] | T[k+132]; j<33 of each row
                # block is used, j reads stay inside the block's 37 planes
                LZ = F3 - 4 * PB
                B = pool.tile([128, F3], u8, tag="B")
                nc.vector.tensor_tensor(
                    out=B[:NP, 0:LZ], in0=T[:NP, 0:LZ],
                    in1=T[:NP, 2 * PB:LZ + 2 * PB], op=OR)
                nc.vector.tensor_tensor(
                    out=B[:NP, 0:LZ], in0=B[:NP, 0:LZ],
                    in1=T[:NP, 4 * PB:LZ + 4 * PB], op=OR)
                # dil flat = z*8580 + Y*33 + b;  Y = p + 65*v
                for v in range(NV):
                    nc.sync.dma_start(
                        out=bass.AP(dil, 65 * v * 33,
                                    [[33, NP], [8580, ZPL], [1, 33]]),
                        in_=B[:NP, v * F:v * F + BUSE],
                    )
    nc.compile()
    _NC_CACHE[key] = nc
    return nc


def _shard_inputs(coords):
    # padded occupancy volume (+4 origins), packed along x
    vol = np.zeros((260, 268, 264), np.uint8)  # z-dim 268: core 7 slab end
    vol[coords[:, 1] + 4, coords[:, 0] + 4, coords[:, 2] + 4] = 1
    volp = np.packbits(vol, axis=-1, bitorder="little")  # [260, 268, 33]
    in_maps = []
    for c in range(8):
        slab = np.zeros((ROWS, ROWB), np.uint8)
        slab[:260] = np.ascontiguousarray(
            volp[:, 33 * c:33 * c + ZB, :]).reshape(260, ROWB)
        in_maps.append({"gridin": slab})
    return in_maps


_LAST_TIMES = {}


def _unshard(dils):
    from concurrent.futures import ThreadPoolExecutor

    def _keys(c):
        npl = min(ZPL, 260 - ZPL * c)
        packed = np.asarray(dils[c])[: npl * 260, :]
        bits = np.unpackbits(packed, axis=1, bitorder="little").reshape(-1)
        return np.flatnonzero(bits).astype(np.int32) + np.int32(
            ZPL * c * (260 * 264))

    with ThreadPoolExecutor(8) as ex:
        keys = list(ex.map(_keys, range(8)))
    sizes = [k.size for k in keys]
    offs = np.concatenate([[0], np.cumsum(sizes)])
    total = int(offs[-1])
    out = np.empty((OUT_ROWS, 4), np.int32)

    def _fill(c):
        # per-core keys are ascending and core key ranges are disjoint and
        # increasing, so each core owns a contiguous slice of the output
        k = keys[c]
        body = out[offs[c]:offs[c + 1]]
        r, x = np.divmod(k, np.int32(264))
        zq, yy = np.divmod(r, np.int32(260))
        body[:, 0] = zq
        body[:, 1] = yy
        body[:, 2] = x
        body[:, 0:3] -= np.int32(2)
        body[:, 3] = 0

    with ThreadPoolExecutor(8) as ex:
        list(ex.map(_fill, range(8)))
    out[total:] = FILL
    return out


def kernel(coords, stride):
    import time as _time
    from concourse.bass_utils import run_bass_kernel_spmd

    coords = np.asarray(coords)
    stride = int(np.asarray(stride))
    assert stride == 2, f"kernel hardcodes stride 2, got {stride}"
    assert coords.shape == (N, 4)

    t0 = _time.time()
    nc = _build_nc()
    t1 = _time.time()
    in_maps = _shard_inputs(coords)
    t2 = _time.time()
    res = run_bass_kernel_spmd(nc, in_maps, core_ids=list(range(8)))
    t3 = _time.time()
    _LAST_TIMES.update(build=t1 - t0, shard=t2 - t1, device=t3 - t2)

    out = _unshard([res.results[c]["dil"] for c in range(8)])
    _LAST_TIMES["post"] = _time.time() - t3
    return out



# revision 2
# speedup vs baseline: 4580.0421x; 4580.0421x over previous
"""CoordinateDensification kernel for 8 TRN2 NeuronCores.

Reference semantics: expand 500k int32 coords [N,4] (cols 0-2 in [0,256),
col 3 == 0) by the 27 offsets {-2,0,2}^3 (stride 2), then sorted row-dedup
padded with INT32_MAX to [N*27, 4].

Device algorithm (SPMD over 8 cores, sharded by 33 dilated z-planes/core):
bit-packed occupancy slab in, 3D binary dilation by {-2,0,2}^3 on device,
packed bitmask out.  ~17 large instructions per core:
  - host marshals coords into a bit-packed occupancy slab
    gridin[265, 37*33] per core: bit (x+4)&7 of byte
    [y+4, (z+4-33c)*33 + (x+4)>>3] (little bit order; rows 260+ zero).
    The +4 origins guarantee rows/planes 0..3 and bits 0..3 of each
    plane's first byte are empty, making every cross-byte / cross-block
    carry in the shifted ORs below provably zero.
  - y-rows mapped row = p + 65*v (65 partitions, v in {0..3} free
    blocks): the whole 260-row slab is ONE pipeline.  y-dilation: the
    slab is loaded 3x at row offsets +0/+2/+4 (DVE cannot read at a
    partition offset; DMA can) and OR'd.
  - x-dilation: fused shift-OR ops in packed-bit space
    (U | U>>2 | U>>4 | next_byte<<6 | next_byte<<4).
  - z-dilation: shifted ORs at 66/132-byte strides.
  - output dil[z*260+Y, 33] is already the packed bitmask; bitmask cell
    order == lexicographic row order of the reference output, so no sort
    is ever needed.  Host unpacks + pads (gather/unshard).

Runner: one jit(shard_map(bass_exec)) built ONCE and cached for the
process (run_bass_kernel_spmd rebuilds the jit closure per call, paying
retrace + re-lower + executable reload over the axon link every call).
The zero-initialized donated output buffers that run_bass_via_pjrt ships
are also dropped: this kernel writes every byte of dil, so the
uninitialized PJRT result buffer is fine and 2.27MB of upload per call
disappears.  Host post-processing is single-threaded (1-CPU container)
and chunked for cache locality.
"""
import sys
sys.path.insert(0, '/opt/trn_rl_repo')
import numpy as np

N = 500000
ZPL = 33                 # dilated planes owned per core
ZB = 37                  # z-slab planes per core (33 + 2 halo each side)
PB = 33                  # packed bytes per plane-row (264 bits)
ROWB = ZB * PB           # 1221 bytes per y-row
ROWS = 265               # 260 y-rows + 4 zero rows for the +2/+4 reads
NP = 65                  # partitions used; row = p + 65*v
NV = 4                   # row blocks per partition (65*4 == 260)
F3 = NV * ROWB           # 4884: per-partition free size
BUSE = ZPL * PB          # 1089: bytes per row after z-dilate
FILL = np.int32(np.iinfo(np.int32).max)
OUT_ROWS = N * 27

_CACHE = {}


def _build_nc(num_devices=8, repeats=1):
    """repeats > 1 duplicates the body back-to-back in one NEFF; used by
    test.py to measure per-iteration HW time as a wall-clock delta."""
    key = ("nc", num_devices, repeats)
    if key in _CACHE:
        return _CACHE[key]
    import concourse.bass as bass
    import concourse.bacc as bacc
    import concourse.tile as tile
    from concourse import mybir

    u8 = mybir.dt.uint8
    OR = mybir.AluOpType.bitwise_or
    SHR = mybir.AluOpType.logical_shift_right
    SHL = mybir.AluOpType.logical_shift_left
    F = ROWB

    nc = bacc.Bacc("TRN2", target_bir_lowering=False, num_devices=num_devices)
    gridin = nc.dram_tensor("gridin", [ROWS, ROWB], u8, kind="ExternalInput")
    dil = nc.dram_tensor("dil", [ZPL * 260, 33], u8, kind="ExternalOutput")

    with tile.TileContext(nc) as tc:
        with tc.tile_pool(name="sbuf", bufs=2) as pool:
            # shift amounts as u8 tiles: bitvec stt ops reject float
            # immediates and need scalars of the operand dtype
            c2 = pool.tile([128, 1], u8, tag="c2")
            nc.vector.memset(c2[:], 2)
            c4 = pool.tile([128, 1], u8, tag="c4")
            nc.vector.memset(c4[:], 4)
            c6 = pool.tile([128, 1], u8, tag="c6")
            nc.vector.memset(c6[:], 6)

            for _rep in range(repeats):
                # rows p+65v per partition; +2/+4 y-shifts via DMA offset
                R0 = pool.tile([128, F3], u8, tag="R0")
                nc.sync.dma_start(
                    out=R0[:NP, :],
                    in_=bass.AP(gridin, 0, [[F, NP], [NP * F, NV], [1, F]]))
                R2 = pool.tile([128, F3], u8, tag="R2")
                nc.scalar.dma_start(
                    out=R2[:NP, :],
                    in_=bass.AP(gridin, 2 * F,
                                [[F, NP], [NP * F, NV], [1, F]]))
                R4 = pool.tile([128, F3], u8, tag="R4")
                nc.gpsimd.dma_start(
                    out=R4[:NP, :],
                    in_=bass.AP(gridin, 4 * F,
                                [[F, NP], [NP * F, NV], [1, F]]))
                # Y
                U = pool.tile([128, F3], u8, tag="U")
                nc.vector.tensor_tensor(
                    out=U[:NP, :], in0=R0[:NP, :], in1=R2[:NP, :], op=OR)
                nc.vector.tensor_tensor(
                    out=U[:NP, :], in0=U[:NP, :], in1=R4[:NP, :], op=OR)
                # X: T = U | U>>2 | U>>4 | next<<6 | next<<4
                T = pool.tile([128, F3], u8, tag="T")
                nc.vector.scalar_tensor_tensor(
                    out=T[:NP, :], in0=U[:NP, :], scalar=c2[:NP, :],
                    in1=U[:NP, :], op0=SHR, op1=OR)
                nc.vector.scalar_tensor_tensor(
                    out=T[:NP, :], in0=U[:NP, :], scalar=c4[:NP, :],
                    in1=T[:NP, :], op0=SHR, op1=OR)
                nc.vector.scalar_tensor_tensor(
                    out=T[:NP, 0:F3 - 1], in0=U[:NP, 1:F3], scalar=c6[:NP, :],
                    in1=T[:NP, 0:F3 - 1], op0=SHL, op1=OR)
                nc.vector.scalar_tensor_tensor(
                    out=T[:NP, 0:F3 - 1], in0=U[:NP, 1:F3], scalar=c4[:NP, :],
                    in1=T[:NP, 0:F3 - 1], op0=SHL, op1=OR)
                # Z: B[k] = T[k] | T[k+66] | T[k+132]; j<33 of each row
                # block is used, j reads stay inside the block's 37 planes
                LZ = F3 - 4 * PB
                B = pool.tile([128, F3], u8, tag="B")
                nc.vector.tensor_tensor(
                    out=B[:NP, 0:LZ], in0=T[:NP, 0:LZ],
                    in1=T[:NP, 2 * PB:LZ + 2 * PB], op=OR)
                nc.vector.tensor_tensor(
                    out=B[:NP, 0:LZ], in0=B[:NP, 0:LZ],
                    in1=T[:NP, 4 * PB:LZ + 4 * PB], op=OR)
                # dil flat = z*8580 + Y*33 + b;  Y = p + 65*v
                for v in range(NV):
                    nc.sync.dma_start(
                        out=bass.AP(dil, 65 * v * 33,
                                    [[33, NP], [8580, ZPL], [1, 33]]),
                        in_=B[:NP, v * F:v * F + BUSE],
                    )
    nc.compile()
    _CACHE[key] = nc
    return nc


def _make_runner(repeats=1):
    """jit(shard_map(bass_exec)) over the 8 cores, cached per `repeats`.

    No donated zero output buffers: the kernel writes every byte of dil,
    so the uninitialized PJRT result buffer is safe and the 283KB/core
    zeros upload is skipped.
    """
    key = ("runner", repeats)
    if key in _CACHE:
        return _CACHE[key]
    import jax
    from jax.sharding import Mesh, PartitionSpec
    from jax.experimental.shard_map import shard_map
    from concourse.bass2jax import (
        _bass_exec_p, install_neuronx_cc_hook, partition_id_tensor)

    install_neuronx_cc_hook()
    nc = _build_nc(repeats=repeats)
    pname = nc.partition_id_tensor.name if nc.partition_id_tensor else None
    out_avals = [jax.core.ShapedArray((ZPL * 260, 33), np.uint8)]

    def _body(gridin):
        operands = [gridin]
        if pname is not None:
            operands.append(partition_id_tensor())
        return tuple(_bass_exec_p.bind(
            *operands,
            out_avals=tuple(out_avals),
            in_names=("gridin", pname) if pname else ("gridin",),
            out_names=("dil",),
            lowering_input_output_aliases=(),
            sim_require_finite=True,
            sim_require_nnan=True,
            nc=nc,
        ))

    devices = jax.devices()[:8]
    mesh = Mesh(np.asarray(devices), ("core",))
    sharded = jax.jit(
        shard_map(_body, mesh=mesh, in_specs=(PartitionSpec("core"),),
                  out_specs=(PartitionSpec("core"),), check_rep=False),
        keep_unused=True,
    )
    _CACHE[key] = sharded
    return sharded


def _shard_inputs(coords):
    # padded occupancy volume (+4 origins), packed along x; one concat
    # array whose axis-0 shards are the per-core slabs
    vol = np.zeros((260, 268, 264), np.uint8)  # z-dim 268: core 7 slab end
    vol[coords[:, 1] + 4, coords[:, 0] + 4, coords[:, 2] + 4] = 1
    volp = np.packbits(vol, axis=-1, bitorder="little")  # [260, 268, 33]
    concat = np.zeros((8 * ROWS, ROWB), np.uint8)
    for c in range(8):
        concat[c * ROWS:c * ROWS + 260] = np.ascontiguousarray(
            volp[:, 33 * c:33 * c + ZB, :]).reshape(260, ROWB)
    return concat


def _unshard(dils):
    """dils: [8, ZPL*260, 33] packed bitmasks -> full [N*27, 4] output.

    Single-threaded (1-CPU container; threads only add overhead) with
    chunked decode so the divmod temporaries stay cache-resident.
    """
    out = np.empty((OUT_ROWS, 4), np.int32)
    pos = 0
    CH = 1 << 17
    for c in range(8):
        npl = min(ZPL, 260 - ZPL * c)
        bits = np.unpackbits(
            dils[c][: npl * 260, :], axis=1, bitorder="little").reshape(-1)
        k = np.flatnonzero(bits).astype(np.int32)
        k += np.int32(ZPL * c * (260 * 264))
        n = k.size
        # per-core keys are ascending and core key ranges are disjoint and
        # increasing, so each core owns a contiguous slice of the output
        for s in range(0, n, CH):
            kk = k[s:s + CH]
            r, x = np.divmod(kk, np.int32(264))
            zq, yy = np.divmod(r, np.int32(260))
            body = out[pos + s:pos + s + kk.size]
            body[:, 0] = zq - np.int32(2)
            body[:, 1] = yy - np.int32(2)
            body[:, 2] = x - np.int32(2)
            body[:, 3] = 0
        pos += n
    out[pos:] = FILL
    return out


_LAST_TIMES = {}


def kernel(coords, stride):
    import time as _time

    coords = np.asarray(coords)
    stride = int(np.asarray(stride))
    assert stride == 2, f"kernel hardcodes stride 2, got {stride}"
    assert coords.shape == (N, 4)

    t0 = _time.time()
    runner = _make_runner()
    t1 = _time.time()
    concat = _shard_inputs(coords)
    t2 = _time.time()
    dil = np.asarray(runner(concat)[0]).reshape(8, ZPL * 260, 33)
    t3 = _time.time()
    out = _unshard(dil)
    t4 = _time.time()
    _LAST_TIMES.update(build=t1 - t0, shard=t2 - t1, device=t3 - t2,
                       post=t4 - t3)
    return out


def measure_hw_exec_ns(coords, r_lo=8, r_hi=264, trials=7):
    """Per-iteration on-device execution time via the repeats delta.

    NTFF/neuron-profile is unavailable under this axon client (no
    antenv.axon_hooks), so estimate HW time the standard way: one NEFF
    with the kernel body repeated r_lo times, one with r_hi, identical
    I/O.  (wall(r_hi) - wall(r_lo)) / (r_hi - r_lo) cancels the axon
    dispatch/transfer overhead exactly.  Inputs are pre-placed on device
    and outputs are not fetched, so the timed region is dispatch + NEFF
    execution only.
    """
    import time as _time
    import jax
    from jax.sharding import Mesh, PartitionSpec, NamedSharding

    concat = _shard_inputs(np.asarray(coords))
    lo = _make_runner(r_lo)
    hi = _make_runner(r_hi)
    devices = jax.devices()[:8]
    mesh = Mesh(np.asarray(devices), ("core",))
    dev_in = jax.device_put(concat, NamedSharding(mesh, PartitionSpec("core")))

    def mintime(fn):
        fn(dev_in)[0].block_until_ready()  # compile + warm
        ts = []
        for _ in range(trials):
            t0 = _time.perf_counter()
            fn(dev_in)[0].block_until_ready()
            ts.append(_time.perf_counter() - t0)
        return min(ts)

    t_lo = mintime(lo)
    t_hi = mintime(hi)
    per_iter = (t_hi - t_lo) / (r_hi - r_lo)
    return max(1, int(per_iter * 1e9)), t_lo, t_hi
